# revision 30
# baseline (speedup 1.0000x reference)
"""Causal GQA self-attention (B=4, T=2048, C=2048, H=16, HKV=4, D=128) on 8 trn2 cores.

Sharding: core = (batch b = core//2) x (kv-head pair s = core%2).
Each core computes, for its batch and its 2 kv heads (8 q heads):
  q^T = Wq_s^T x^T, k^T, v  (float32r matmuls, full PE rate at N>=256)
  flash-style causal attention in transposed layout (S^T blocks [tk=128, tq=512]),
  rowsums via ones-matmul, per-head late normalization,
  partial c_proj (row-slice of Wc) -> [T, C] partial output.
Host sums the two partials per batch and adds bc.
"""

import math
from contextlib import ExitStack

import numpy as np

B, T, C = 4, 2048, 2048
HKV, D, G = 4, 128, 4
NCORES = 8
HPC = 8            # q heads per core
KVPC = 2           # kv heads per core
TQ = 512           # q-tile (free dim of S^T blocks)
NTQ = T // TQ      # 4
NKB = T // 128     # 16 k-blocks
SCALE = 1.0 / math.sqrt(D)
NEG = -3.0e38

_NC = None


def _round_f32r(a):
    """Round fp32 to f32r (8-bit exp, 11-bit mantissa) to match PE input rounding."""
    u = np.ascontiguousarray(a, dtype=np.float32).view(np.uint32)
    add = ((u >> np.uint32(12)) & np.uint32(1)) + np.uint32(0x7FF)
    u = (u + add) & np.uint32(0xFFFFF000)
    return u.view(np.float32)


def _make_masks():
    import ml_dtypes
    masks = np.zeros((4, 128, TQ), dtype=np.float32)
    i = np.arange(128)[:, None]
    j = np.arange(TQ)[None, :]
    for p in range(4):
        masks[p] = np.where(j >= i + 128 * p, 0.0, NEG)
    return masks.astype(ml_dtypes.bfloat16)


def _emit(tc, io):
    from concourse import mybir

    nc = tc.nc
    F32 = mybir.dt.float32
    F32R = mybir.dt.float32r
    BF16 = mybir.dt.bfloat16
    EXP = mybir.ActivationFunctionType.Exp
    ADD = mybir.AluOpType.add
    MULT = mybir.AluOpType.mult

    ctx = ExitStack()
    with ctx:
        persist = ctx.enter_context(tc.tile_pool(name="persist", bufs=1))
        drampool = ctx.enter_context(tc.tile_pool(name="dram", bufs=1, space="DRAM"))

        kT_sb = persist.tile([128, KVPC * T], F32R, name="kT", tag="kT")   # [d, kv*T + t]
        v_sb = persist.tile([128, NKB * 256], F32R, name="v", tag="v")     # [t%128, tb*256 + kv*128 + d]
        ones_sb = persist.tile([128, 1], F32R, name="ones", tag="ones")
        nc.sync.dma_start(ones_sb[:], io["ones"].bitcast(F32R))
        qT_dram = drampool.tile([HPC, 128, T], F32R, name="qTd", tag="qTd")
        maskp = ctx.enter_context(tc.tile_pool(name="maskp", bufs=1))
        qhpool = ctx.enter_context(tc.tile_pool(name="qh", bufs=2))
        mask_sb = maskp.tile([128, 4 * TQ], BF16, name="mask", tag="mask")
        for p in range(4):
            nc.gpsimd.dma_start(mask_sb[:, p * TQ:(p + 1) * TQ], io["masks"][p])

        def load_q(i):
            hh, qq = divmod(i, NTQ)
            t = qhpool.tile([128, TQ], F32R, name="qTh", tag="qTh", bufs=2)
            nc.gpsimd.dma_start(t[:], qT_dram[hh, :, qq * TQ:(qq + 1) * TQ])
            return t
        rsinv_dram = drampool.tile([HPC * NTQ, TQ], F32, name="rsinvd", tag="rsinvd")

        # ---------------- Phase A: projections ----------------
        with (
            tc.tile_pool(name="xpool", bufs=16) as xpool,
            tc.tile_pool(name="wkres", bufs=1) as wkres,
            tc.tile_pool(name="wstream", bufs=3) as wstream,
            tc.tile_pool(name="stage", bufs=2) as stage,
        ):
            wq3d = io["wq"].rearrange("(cb r) c -> r cb c", r=128)   # [128, 16, 1024]
            wqbs = {}

            def load_wq(hd):
                t = wstream.tile([128, 16, 128], F32R, name="wqb", tag="wqb", bufs=2)
                nc.sync.dma_start(
                    t[:], wq3d[:, :, hd * 128:(hd + 1) * 128].bitcast(F32R))
                return t

            x_sb = []
            _wq_sched = {0: 0, 11: 1}
            for cb in range(16):
                xt = xpool.tile([128, T], F32R, name="x", tag="x")
                nc.sync.dma_start(xt[:], io["xT"][cb * 128:(cb + 1) * 128, :].bitcast(F32R))
                x_sb.append(xt)
                if cb in _wq_sched:
                    hd = _wq_sched[cb]
                    wqbs[hd] = load_wq(hd)
            for hd in range(2, HPC):
                wqbs[hd] = load_wq(hd)
            wk_sb = wkres.tile([128, 16 * 256], F32R, name="wk", tag="wk")
            nc.sync.dma_start(
                wk_sb[:].rearrange("r (cb c) -> r cb c", c=256),
                io["wk"].rearrange("(cb r) c -> r cb c", r=128).bitcast(F32R))
            ident = wkres.tile([128, 128], F32R, name="ident", tag="ident")
            nc.sync.dma_start(ident[:], io["ident"].bitcast(F32R))

            # q projection: q^T[hd, t] ; write to DRAM scratch
            psAq_ctx = tc.tile_pool(name="psAq", bufs=4, space="PSUM")
            psAq = psAq_ctx.__enter__()
            for hd in range(HPC):
                wqb = wqbs.pop(hd)
                for t4g in range(2):
                    ps_q = [psAq.tile([128, TQ], F32, name="qps", tag="qps")
                            for _ in range(2)]
                    for cb in range(16):
                        for t4i in range(2):
                            t4 = t4g * 2 + t4i
                            nc.tensor.matmul(ps_q[t4i][:],
                                             lhsT=wqb[:, cb, :],
                                             rhs=x_sb[cb][:, t4 * TQ:(t4 + 1) * TQ],
                                             start=(cb == 0), stop=(cb == 15))
                    for t4i in range(2):
                        t4 = t4g * 2 + t4i
                        qst = stage.tile([128, TQ], F32R, name="qstage", tag="stg", bufs=2)
                        nc.vector.tensor_copy(qst[:], ps_q[t4i][:])
                        nc.scalar.dma_start(qT_dram[hd, :, t4 * TQ:(t4 + 1) * TQ], qst[:])

            psAq_ctx.__exit__(None, None, None)

            # k projection: k^T[d, t] resident
            psAk_ctx = tc.tile_pool(name="psAk", bufs=2, space="PSUM")
            psAk = psAk_ctx.__enter__()
            psAv_ctx = tc.tile_pool(name="psAv", bufs=4, space="PSUM")
            psAv = psAv_ctx.__enter__()
            psAt_ctx = tc.tile_pool(name="psAt", bufs=2, space="PSUM")
            psAt = psAt_ctx.__enter__()
            for kv in range(KVPC):
                for t4 in range(NTQ):
                    ps_k = psAk.tile([128, TQ], F32, name="kps", tag="kps")
                    for cb in range(16):
                        nc.tensor.matmul(
                            ps_k[:],
                            lhsT=wk_sb[:, cb * 256 + kv * 128:cb * 256 + (kv + 1) * 128],
                            rhs=x_sb[cb][:, t4 * TQ:(t4 + 1) * TQ],
                            start=(cb == 0), stop=(cb == 15))
                    nc.vector.tensor_copy(
                        kT_sb[:, kv * T + t4 * TQ:kv * T + (t4 + 1) * TQ], ps_k[:])

            # v projection via transpose path:
            # v^T[vd, t] (N=512 matmuls), then PE-transpose to v[t, vd]
            for grp in range(2):
                ps_vT = [psAv.tile([128, TQ], F32, name="vTps", tag="vTps")
                         for _ in range(4)]
                for cb in range(16):
                    wv_t = wstream.tile([128, 256], F32R, name="wv", tag="wv", bufs=2)
                    nc.sync.dma_start(wv_t[:],
                                      io["wv"][cb * 128:(cb + 1) * 128, :].bitcast(F32R))
                    for i in range(4):
                        kv, t4 = divmod(grp * 4 + i, NTQ)
                        nc.tensor.matmul(ps_vT[i][:],
                                         lhsT=wv_t[:, kv * 128:(kv + 1) * 128],
                                         rhs=x_sb[cb][:, t4 * TQ:(t4 + 1) * TQ],
                                         start=(cb == 0), stop=(cb == 15))
                for i in range(4):
                    kv, t4 = divmod(grp * 4 + i, NTQ)
                    vT_st = stage.tile([128, TQ], F32R, name="vTst", tag="stg", bufs=2)
                    nc.vector.tensor_copy(vT_st[:], ps_vT[i][:])
                    for sub in range(4):          # transpose [vd=128, t=128] -> [t, vd]
                        tb = t4 * 4 + sub
                        ps_t = psAt.tile([128, 128], F32R, name="tps", tag="tps")
                        nc.tensor.transpose(ps_t[:],
                                            vT_st[:, sub * 128:(sub + 1) * 128],
                                            ident[:])
                        nc.vector.tensor_copy(
                            v_sb[:, tb * 256 + kv * 128:tb * 256 + (kv + 1) * 128],
                            ps_t[:])

            psAt_ctx.__exit__(None, None, None)
            psAv_ctx.__exit__(None, None, None)
            psAk_ctx.__exit__(None, None, None)

        # ---------------- Phase B: attention (+ wc prefetch) ----------------
        yres = ctx.enter_context(tc.tile_pool(name="yres", bufs=8))
        yT_all = [yres.tile([128, T], F32R, name="yT", tag="yT") for _ in range(HPC)]
        wcres = ctx.enter_context(tc.tile_pool(name="wcres", bufs=1))
        wc_sb = [wcres.tile([128, C], F32R, name="wc", tag=f"wc{h}")
                 for h in range(HPC)]

        with (
            tc.tile_pool(name="pTp", bufs=3) as pTpool,
            tc.tile_pool(name="smp", bufs=2) as smpool,
            tc.tile_pool(name="normp", bufs=1) as normp,
            tc.tile_pool(name="binvp", bufs=2) as binvpool,
            tc.tile_pool(name="psg", bufs=2, space="PSUM") as psg,
            tc.tile_pool(name="psy", bufs=2, space="PSUM") as psy,
            tc.tile_pool(name="psr", bufs=2, space="PSUM") as psr,
        ):
            rsh_all = [normp.tile([16, 128], F32, name="rsh", tag=f"rsh{h}",
                                  bufs=1) for h in range(HPC)]

            def emit_norm(hh):
                # per-head normalization, overlapped with later attention
                rsinv_h = normp.tile([16, 128], F32, name="rsinvh", tag="rsinvh",
                                     bufs=2)
                nc.vector.reciprocal(rsinv_h[:], rsh_all[hh][:])
                nc.gpsimd.dma_start(
                    rsinv_dram[hh * NTQ:(hh + 1) * NTQ, :]
                    .rearrange("q (p j) -> (q p) j", j=128),
                    rsinv_h[:])
                for qq in range(NTQ):
                    binv = binvpool.tile([128, TQ], F32, name="binv", tag="binv")
                    nc.gpsimd.dma_start(
                        binv[:],
                        rsinv_dram[hh * NTQ + qq:hh * NTQ + qq + 1, :].to_broadcast([128, TQ]))
                    nc.vector.tensor_tensor(
                        out=yT_all[hh][:, qq * TQ:(qq + 1) * TQ],
                        in0=yT_all[hh][:, qq * TQ:(qq + 1) * TQ],
                        in1=binv[:],
                        op=MULT)
                nc.gpsimd.dma_start(wc_sb[hh][:],
                                    io["wc"][hh * 128:(hh + 1) * 128, :].bitcast(F32R))

            def emit_S(g, qT_h, kv, qt):
                sg = psg.tile([128, 2 * TQ], F32, name="sg", tag="sg")
                for j in range(2):
                    kb = g * 2 + j
                    nc.tensor.matmul(
                        sg[:, j * TQ:(j + 1) * TQ],
                        lhsT=kT_sb[:, kv * T + kb * 128:kv * T + (kb + 1) * 128],
                        rhs=qT_h[:],
                        start=True, stop=True)
                return sg

            def emit_exp(g, sg, qt):
                pT = pTpool.tile([128, 2 * TQ], F32R, name="pT", tag="pT")
                if g * 2 + 2 > qt * (TQ // 128):   # crossing group
                    smg = smpool.tile([128, 2 * TQ], F32, name="sm", tag="sm")
                    for j in range(2):
                        kb = g * 2 + j
                        p = kb - qt * (TQ // 128)
                        nc.vector.tensor_tensor(
                            out=smg[:, j * TQ:(j + 1) * TQ],
                            in0=sg[:, j * TQ:(j + 1) * TQ],
                            in1=mask_sb[:, p * TQ:(p + 1) * TQ],
                            op=ADD)
                    nc.scalar.activation(pT[:], smg[:], EXP, scale=SCALE)
                else:
                    nc.scalar.activation(pT[:], sg[:], EXP, scale=SCALE)
                return pT

            def emit_AV(p):
                for j in range(2):
                    kb = p["g"] * 2 + j
                    kv_ = p["kv"]
                    nc.tensor.matmul(
                        p["y"][:],
                        lhsT=v_sb[:, kb * 256 + kv_ * 128:kb * 256 + (kv_ + 1) * 128],
                        rhs=p["pT"][:, j * TQ:(j + 1) * TQ],
                        start=(kb == 0), stop=(kb == p["nkb"] - 1))
                    nc.tensor.matmul(
                        p["rs"][:],
                        lhsT=ones_sb[:],
                        rhs=p["pT"][:, j * TQ:(j + 1) * TQ],
                        start=(kb == 0), stop=(kb == p["nkb"] - 1))
                if p["lastg"]:
                    hh, qq = p["h"], p["qt"]
                    nc.vector.tensor_copy(yT_all[hh][:, qq * TQ:(qq + 1) * TQ],
                                          p["y"][:])
                    rs_st = smpool.tile([1, TQ], F32, name="rsst", tag="rsst", bufs=3)
                    nc.scalar.copy(rs_st[:], p["rs"][:])
                    nc.gpsimd.dma_start(rsh_all[hh][qq * 4:(qq + 1) * 4, :], rs_st[:])

            pending = None
            q_next = load_q(0)
            for h in range(HPC):
                kv = h // G
                for qt in range(NTQ):
                    qT_h = q_next
                    if h * NTQ + qt + 1 < HPC * NTQ:
                        q_next = load_q(h * NTQ + qt + 1)
                    nkb = (qt + 1) * (TQ // 128)
                    ng = nkb // 2           # groups of 2 k-blocks
                    y_ps = psy.tile([128, TQ], F32, name="yps", tag="yps")
                    rs_ps = psr.tile([1, TQ], F32, name="rsps", tag="rsps")
                    for g in range(ng):
                        sg = emit_S(g, qT_h, kv, qt)
                        if pending is not None:
                            emit_AV(pending)
                        pT = emit_exp(g, sg, qt)
                        pending = {"pT": pT, "g": g, "kv": kv, "nkb": nkb,
                                   "y": y_ps, "rs": rs_ps,
                                   "lastg": (g == ng - 1), "h": h, "qt": qt}
                    if qt == 2 and h > 0:
                        emit_norm(h - 1)
            emit_AV(pending)
            emit_norm(HPC - 1)

        # ---------------- Phase C: output projection ----------------
        with (
            tc.tile_pool(name="ostage", bufs=2) as ostage,
            tc.tile_pool(name="psC", bufs=8, space="PSUM") as psC,
        ):
            for tq in range(16):
                ops = [psC.tile([128, 512], F32, name="ops", tag="ops") for _ in range(4)]
                for h in range(HPC):
                    for cp in range(4):
                        nc.tensor.matmul(
                            ops[cp][:],
                            lhsT=yT_all[h][:, tq * 128:(tq + 1) * 128],
                            rhs=wc_sb[h][:, cp * 512:(cp + 1) * 512],
                            start=(h == 0), stop=(h == HPC - 1))
                ost = ostage.tile([128, C], F32, name="ost", tag="ost")
                for cp in range(4):
                    nc.scalar.copy(ost[:, cp * 512:(cp + 1) * 512], ops[cp][:])
                nc.scalar.dma_start(io["out"][tq * 128:(tq + 1) * 128, :], ost[:])


def _build_nc():
    import concourse.tile as tile
    from concourse import bacc, mybir

    F32 = mybir.dt.float32
    BF16 = mybir.dt.bfloat16
    nc = bacc.Bacc("TRN2", target_bir_lowering=False, debug=False,
                   num_devices=NCORES)
    io = {
        "xT": nc.dram_tensor("xT", [C, T], F32, kind="ExternalInput").ap(),
        "wq": nc.dram_tensor("wq", [C, HPC * 128], F32, kind="ExternalInput").ap(),
        "wk": nc.dram_tensor("wk", [C, KVPC * 128], F32, kind="ExternalInput").ap(),
        "wv": nc.dram_tensor("wv", [C, KVPC * 128], F32, kind="ExternalInput").ap(),
        "wc": nc.dram_tensor("wc", [HPC * 128, C], F32, kind="ExternalInput").ap(),
        "masks": nc.dram_tensor("masks", [4, 128, TQ], BF16, kind="ExternalInput").ap(),
        "ones": nc.dram_tensor("ones", [128, 1], F32, kind="ExternalInput").ap(),
        "ident": nc.dram_tensor("ident", [128, 128], F32, kind="ExternalInput").ap(),
        "out": nc.dram_tensor("out", [T, C], F32, kind="ExternalOutput").ap(),
    }
    with tile.TileContext(nc) as tc:
        _emit(tc, io)
    nc.compile()
    return nc


def _get_nc():
    global _NC
    if _NC is None:
        _NC = _build_nc()
    return _NC


def make_in_maps(x, Wq, Wkv, Wc):
    x = np.asarray(x, dtype=np.float32)
    Wq = np.asarray(Wq, dtype=np.float32)
    Wkv = np.asarray(Wkv, dtype=np.float32)
    Wc = np.asarray(Wc, dtype=np.float32)
    masks = _make_masks()
    ones = np.ones((128, 1), dtype=np.float32)
    in_maps = []
    for core in range(NCORES):
        b, s = core // 2, core % 2
        in_maps.append({
            "xT": _round_f32r(x[b].T),
            "wq": _round_f32r(Wq[:, s * 1024:(s + 1) * 1024]),
            "wk": _round_f32r(Wkv[:, s * 256:(s + 1) * 256]),
            "wv": _round_f32r(Wkv[:, 512 + s * 256:512 + (s + 1) * 256]),
            "wc": _round_f32r(Wc[s * 1024:(s + 1) * 1024, :]),
            "masks": masks,
            "ones": ones,
            "ident": np.eye(128, dtype=np.float32),
        })
    return in_maps


def combine_outputs(results, bc):
    bc = np.asarray(bc, dtype=np.float32)
    out = np.empty((B, T, C), dtype=np.float32)
    for b in range(B):
        out[b] = results[2 * b]["out"] + results[2 * b + 1]["out"]
    out += bc[None, None, :]
    return out


def kernel(x, Wq, Wkv, Wc, bc):
    from concourse.bass_utils import run_bass_kernel_spmd

    nc = _get_nc()
    in_maps = make_in_maps(x, Wq, Wkv, Wc)
    res = run_bass_kernel_spmd(nc, in_maps, list(range(NCORES)))
    return combine_outputs(res.results, bc)


# revision 31
# speedup vs baseline: 1.0633x; 1.0633x over previous
"""Causal GQA self-attention (B=4, T=2048, C=2048, H=16, HKV=4, D=128) on 8 trn2 cores.

Sharding: core = (batch b = core//2) x (kv-head pair s = core%2).
Each core computes, for its batch and its 2 kv heads (8 q heads):
  q^T = Wq_s^T x^T, k^T, v  (float32r matmuls, full PE rate at N>=256)
  flash-style causal attention in transposed layout (S^T blocks [tk=128, tq=512]),
  rowsums via ones-matmul, per-head late normalization,
  partial c_proj (row-slice of Wc) -> [T, C] partial output.
Host sums the two partials per batch and adds bc.
"""

import math
from contextlib import ExitStack

import numpy as np

B, T, C = 4, 2048, 2048
HKV, D, G = 4, 128, 4
NCORES = 8
HPC = 8            # q heads per core
KVPC = 2           # kv heads per core
TQ = 512           # q-tile (free dim of S^T blocks)
NTQ = T // TQ      # 4
NKB = T // 128     # 16 k-blocks
SCALE = 1.0 / math.sqrt(D)
NEG = -3.0e38

_NC = None


def _round_f32r(a):
    """Round fp32 to f32r (8-bit exp, 11-bit mantissa) to match PE input rounding."""
    u = np.ascontiguousarray(a, dtype=np.float32).view(np.uint32)
    add = ((u >> np.uint32(12)) & np.uint32(1)) + np.uint32(0x7FF)
    u = (u + add) & np.uint32(0xFFFFF000)
    return u.view(np.float32)


def _make_masks():
    import ml_dtypes
    masks = np.zeros((4, 128, TQ), dtype=np.float32)
    i = np.arange(128)[:, None]
    j = np.arange(TQ)[None, :]
    for p in range(4):
        masks[p] = np.where(j >= i + 128 * p, 0.0, NEG)
    return masks.astype(ml_dtypes.bfloat16)


def _emit(tc, io):
    from concourse import mybir

    nc = tc.nc
    F32 = mybir.dt.float32
    F32R = mybir.dt.float32r
    BF16 = mybir.dt.bfloat16
    EXP = mybir.ActivationFunctionType.Exp
    ADD = mybir.AluOpType.add
    MULT = mybir.AluOpType.mult

    ctx = ExitStack()
    with ctx:
        persist = ctx.enter_context(tc.tile_pool(name="persist", bufs=1))
        drampool = ctx.enter_context(tc.tile_pool(name="dram", bufs=1, space="DRAM"))

        kT_sb = persist.tile([128, KVPC * T], F32R, name="kT", tag="kT")   # [d, kv*T + t]
        v_sb = persist.tile([128, NKB * 256], F32R, name="v", tag="v")     # [t%128, tb*256 + kv*128 + d]
        ones_sb = persist.tile([128, 1], F32R, name="ones", tag="ones")
        nc.sync.dma_start(ones_sb[:], io["ones"].bitcast(F32R))
        qT_dram = drampool.tile([HPC, 128, T], F32R, name="qTd", tag="qTd")
        maskp = ctx.enter_context(tc.tile_pool(name="maskp", bufs=1))
        qhpool = ctx.enter_context(tc.tile_pool(name="qh", bufs=2))
        mask_sb = maskp.tile([128, 4 * TQ], BF16, name="mask", tag="mask")
        for p in range(4):
            nc.gpsimd.dma_start(mask_sb[:, p * TQ:(p + 1) * TQ], io["masks"][p])

        def load_q(i):
            hh, qq = divmod(i, NTQ)
            t = qhpool.tile([128, TQ], F32R, name="qTh", tag="qTh", bufs=2)
            nc.gpsimd.dma_start(t[:], qT_dram[hh, :, qq * TQ:(qq + 1) * TQ])
            return t
        rsinv_dram = drampool.tile([HPC * NTQ, TQ], F32, name="rsinvd", tag="rsinvd")

        # ---------------- Phase A: projections ----------------
        with (
            tc.tile_pool(name="xpool", bufs=16) as xpool,
            tc.tile_pool(name="wkres", bufs=1) as wkres,
            tc.tile_pool(name="wstream", bufs=3) as wstream,
            tc.tile_pool(name="stage", bufs=2) as stage,
        ):
            wq3d = io["wq"].rearrange("(cb r) c -> r cb c", r=128)   # [128, 16, 1024]
            wqbs = {}

            def load_wq(hd):
                t = wstream.tile([128, 16, 128], F32R, name="wqb", tag="wqb", bufs=2)
                nc.sync.dma_start(
                    t[:], wq3d[:, :, hd * 128:(hd + 1) * 128].bitcast(F32R))
                return t

            x_sb = []
            _wq_sched = {0: 0, 11: 1}
            for cb in range(16):
                xt = xpool.tile([128, T], F32R, name="x", tag="x")
                nc.sync.dma_start(xt[:], io["xT"][cb * 128:(cb + 1) * 128, :].bitcast(F32R))
                x_sb.append(xt)
                if cb in _wq_sched:
                    hd = _wq_sched[cb]
                    wqbs[hd] = load_wq(hd)
            for hd in range(2, HPC):
                wqbs[hd] = load_wq(hd)
            wk_sb = wkres.tile([128, 16 * 256], F32R, name="wk", tag="wk")
            nc.sync.dma_start(
                wk_sb[:].rearrange("r (cb c) -> r cb c", c=256),
                io["wk"].rearrange("(cb r) c -> r cb c", r=128).bitcast(F32R))
            ident = wkres.tile([128, 128], F32R, name="ident", tag="ident")
            nc.sync.dma_start(ident[:], io["ident"].bitcast(F32R))

            # q projection: q^T[hd, t] ; write to DRAM scratch
            psAq_ctx = tc.tile_pool(name="psAq", bufs=4, space="PSUM")
            psAq = psAq_ctx.__enter__()
            for hd in range(HPC):
                wqb = wqbs.pop(hd)
                for t4g in range(2):
                    ps_q = [psAq.tile([128, TQ], F32, name="qps", tag="qps")
                            for _ in range(2)]
                    for cb in range(16):
                        for t4i in range(2):
                            t4 = t4g * 2 + t4i
                            nc.tensor.matmul(ps_q[t4i][:],
                                             lhsT=wqb[:, cb, :],
                                             rhs=x_sb[cb][:, t4 * TQ:(t4 + 1) * TQ],
                                             start=(cb == 0), stop=(cb == 15))
                    for t4i in range(2):
                        t4 = t4g * 2 + t4i
                        qst = stage.tile([128, TQ], F32R, name="qstage", tag="stg", bufs=2)
                        nc.vector.tensor_copy(qst[:], ps_q[t4i][:])
                        nc.scalar.dma_start(qT_dram[hd, :, t4 * TQ:(t4 + 1) * TQ], qst[:])

            psAq_ctx.__exit__(None, None, None)

            # k projection: k^T[d, t] resident
            psAk_ctx = tc.tile_pool(name="psAk", bufs=2, space="PSUM")
            psAk = psAk_ctx.__enter__()
            psAv_ctx = tc.tile_pool(name="psAv", bufs=4, space="PSUM")
            psAv = psAv_ctx.__enter__()
            psAt_ctx = tc.tile_pool(name="psAt", bufs=2, space="PSUM")
            psAt = psAt_ctx.__enter__()
            for kv in range(KVPC):
                for t4 in range(NTQ):
                    ps_k = psAk.tile([128, TQ], F32, name="kps", tag="kps")
                    for cb in range(16):
                        nc.tensor.matmul(
                            ps_k[:],
                            lhsT=wk_sb[:, cb * 256 + kv * 128:cb * 256 + (kv + 1) * 128],
                            rhs=x_sb[cb][:, t4 * TQ:(t4 + 1) * TQ],
                            start=(cb == 0), stop=(cb == 15))
                    nc.vector.tensor_copy(
                        kT_sb[:, kv * T + t4 * TQ:kv * T + (t4 + 1) * TQ], ps_k[:])

            # v projection via transpose path:
            # v^T[vd, t] (N=512 matmuls), then PE-transpose to v[t, vd]
            for grp in range(2):
                ps_vT = [psAv.tile([128, TQ], F32, name="vTps", tag="vTps")
                         for _ in range(4)]
                for cb in range(16):
                    wv_t = wstream.tile([128, 256], F32R, name="wv", tag="wv", bufs=2)
                    nc.sync.dma_start(wv_t[:],
                                      io["wv"][cb * 128:(cb + 1) * 128, :].bitcast(F32R))
                    for i in range(4):
                        kv, t4 = divmod(grp * 4 + i, NTQ)
                        nc.tensor.matmul(ps_vT[i][:],
                                         lhsT=wv_t[:, kv * 128:(kv + 1) * 128],
                                         rhs=x_sb[cb][:, t4 * TQ:(t4 + 1) * TQ],
                                         start=(cb == 0), stop=(cb == 15))
                for i in range(4):
                    kv, t4 = divmod(grp * 4 + i, NTQ)
                    vT_st = stage.tile([128, TQ], F32R, name="vTst", tag="stg", bufs=2)
                    nc.vector.tensor_copy(vT_st[:], ps_vT[i][:])
                    for sub in range(4):          # transpose [vd=128, t=128] -> [t, vd]
                        tb = t4 * 4 + sub
                        ps_t = psAt.tile([128, 128], F32R, name="tps", tag="tps")
                        nc.tensor.transpose(ps_t[:],
                                            vT_st[:, sub * 128:(sub + 1) * 128],
                                            ident[:])
                        nc.vector.tensor_copy(
                            v_sb[:, tb * 256 + kv * 128:tb * 256 + (kv + 1) * 128],
                            ps_t[:])

            psAt_ctx.__exit__(None, None, None)
            psAv_ctx.__exit__(None, None, None)
            psAk_ctx.__exit__(None, None, None)

        # ---------------- Phase B: attention (+ wc prefetch) ----------------
        yres = ctx.enter_context(tc.tile_pool(name="yres", bufs=8))
        yT_all = [yres.tile([128, T], F32R, name="yT", tag="yT") for _ in range(HPC)]
        wcres = ctx.enter_context(tc.tile_pool(name="wcres", bufs=1))
        wc_sb = [wcres.tile([128, C], F32R, name="wc", tag=f"wc{h}")
                 for h in range(HPC)]

        with (
            tc.tile_pool(name="pTp", bufs=3) as pTpool,
            tc.tile_pool(name="smp", bufs=2) as smpool,
            tc.tile_pool(name="normp", bufs=1) as normp,
            tc.tile_pool(name="binvp", bufs=2) as binvpool,
            tc.tile_pool(name="psg", bufs=3, space="PSUM") as psg,
            tc.tile_pool(name="psy", bufs=1, space="PSUM") as psy,
            tc.tile_pool(name="psr", bufs=1, space="PSUM") as psr,
        ):
            rsh_all = [normp.tile([16, 128], F32, name="rsh", tag=f"rsh{h}",
                                  bufs=1) for h in range(HPC)]

            def emit_norm(hh):
                # per-head normalization, overlapped with later attention
                rsinv_h = normp.tile([16, 128], F32, name="rsinvh", tag="rsinvh",
                                     bufs=2)
                nc.vector.reciprocal(rsinv_h[:], rsh_all[hh][:])
                nc.gpsimd.dma_start(
                    rsinv_dram[hh * NTQ:(hh + 1) * NTQ, :]
                    .rearrange("q (p j) -> (q p) j", j=128),
                    rsinv_h[:])
                for qq in range(NTQ):
                    binv = binvpool.tile([128, TQ], F32, name="binv", tag="binv")
                    nc.gpsimd.dma_start(
                        binv[:],
                        rsinv_dram[hh * NTQ + qq:hh * NTQ + qq + 1, :].to_broadcast([128, TQ]))
                    nc.vector.tensor_tensor(
                        out=yT_all[hh][:, qq * TQ:(qq + 1) * TQ],
                        in0=yT_all[hh][:, qq * TQ:(qq + 1) * TQ],
                        in1=binv[:],
                        op=MULT)
                nc.gpsimd.dma_start(wc_sb[hh][:],
                                    io["wc"][hh * 128:(hh + 1) * 128, :].bitcast(F32R))

            def emit_S(g, qT_h, kv, qt):
                sg = psg.tile([128, 2 * TQ], F32, name="sg", tag="sg")
                for j in range(2):
                    kb = g * 2 + j
                    nc.tensor.matmul(
                        sg[:, j * TQ:(j + 1) * TQ],
                        lhsT=kT_sb[:, kv * T + kb * 128:kv * T + (kb + 1) * 128],
                        rhs=qT_h[:],
                        start=True, stop=True)
                return sg

            def emit_exp(g, sg, qt):
                pT = pTpool.tile([128, 2 * TQ], F32R, name="pT", tag="pT")
                if g * 2 + 2 > qt * (TQ // 128):   # crossing group
                    smg = smpool.tile([128, 2 * TQ], F32, name="sm", tag="sm")
                    for j in range(2):
                        kb = g * 2 + j
                        p = kb - qt * (TQ // 128)
                        nc.vector.tensor_tensor(
                            out=smg[:, j * TQ:(j + 1) * TQ],
                            in0=sg[:, j * TQ:(j + 1) * TQ],
                            in1=mask_sb[:, p * TQ:(p + 1) * TQ],
                            op=ADD)
                    nc.scalar.activation(pT[:], smg[:], EXP, scale=SCALE)
                else:
                    nc.scalar.activation(pT[:], sg[:], EXP, scale=SCALE)
                return pT

            def emit_AV(p):
                for j in range(2):
                    kb = p["g"] * 2 + j
                    kv_ = p["kv"]
                    nc.tensor.matmul(
                        p["y"][:],
                        lhsT=v_sb[:, kb * 256 + kv_ * 128:kb * 256 + (kv_ + 1) * 128],
                        rhs=p["pT"][:, j * TQ:(j + 1) * TQ],
                        start=(kb == 0), stop=(kb == p["nkb"] - 1))
                    nc.tensor.matmul(
                        p["rs"][:],
                        lhsT=ones_sb[:],
                        rhs=p["pT"][:, j * TQ:(j + 1) * TQ],
                        start=(kb == 0), stop=(kb == p["nkb"] - 1))
                if p["lastg"]:
                    hh, qq = p["h"], p["qt"]
                    nc.vector.tensor_copy(yT_all[hh][:, qq * TQ:(qq + 1) * TQ],
                                          p["y"][:])
                    rs_st = smpool.tile([1, TQ], F32, name="rsst", tag="rsst", bufs=3)
                    nc.scalar.copy(rs_st[:], p["rs"][:])
                    nc.gpsimd.dma_start(rsh_all[hh][qq * 4:(qq + 1) * 4, :], rs_st[:])

            q_next = load_q(0)
            for h in range(HPC):
                kv = h // G
                for qt in range(NTQ):
                    qT_h = q_next
                    if h * NTQ + qt + 1 < HPC * NTQ:
                        q_next = load_q(h * NTQ + qt + 1)
                    nkb = (qt + 1) * (TQ // 128)
                    ng = nkb // 2           # groups of 2 k-blocks
                    y_ps = psy.tile([128, TQ], F32, name="yps", tag="yps")
                    rs_ps = psr.tile([1, TQ], F32, name="rsps", tag="rsps")
                    sg_prev = emit_S(0, qT_h, kv, qt)
                    pT_prev = emit_exp(0, sg_prev, qt)
                    prev = {"pT": pT_prev, "g": 0, "kv": kv, "nkb": nkb,
                            "y": y_ps, "rs": rs_ps, "lastg": (ng == 1),
                            "h": h, "qt": qt}
                    for g in range(1, ng):
                        sg = emit_S(g, qT_h, kv, qt)
                        emit_AV(prev)
                        pT_prev = emit_exp(g, sg, qt)
                        prev = {"pT": pT_prev, "g": g, "kv": kv, "nkb": nkb,
                                "y": y_ps, "rs": rs_ps, "lastg": (g == ng - 1),
                                "h": h, "qt": qt}
                    emit_AV(prev)
                    if qt == 2 and h > 0:
                        emit_norm(h - 1)
            emit_norm(HPC - 1)

        # ---------------- Phase C: output projection ----------------
        with (
            tc.tile_pool(name="ostage", bufs=2) as ostage,
            tc.tile_pool(name="psC", bufs=8, space="PSUM") as psC,
        ):
            for tq in range(16):
                ops = [psC.tile([128, 512], F32, name="ops", tag="ops") for _ in range(4)]
                for h in range(HPC):
                    for cp in range(4):
                        nc.tensor.matmul(
                            ops[cp][:],
                            lhsT=yT_all[h][:, tq * 128:(tq + 1) * 128],
                            rhs=wc_sb[h][:, cp * 512:(cp + 1) * 512],
                            start=(h == 0), stop=(h == HPC - 1))
                ost = ostage.tile([128, C], F32, name="ost", tag="ost")
                for cp in range(4):
                    nc.scalar.copy(ost[:, cp * 512:(cp + 1) * 512], ops[cp][:])
                nc.scalar.dma_start(io["out"][tq * 128:(tq + 1) * 128, :], ost[:])


def _build_nc():
    import concourse.tile as tile
    from concourse import bacc, mybir

    F32 = mybir.dt.float32
    BF16 = mybir.dt.bfloat16
    nc = bacc.Bacc("TRN2", target_bir_lowering=False, debug=False,
                   num_devices=NCORES)
    io = {
        "xT": nc.dram_tensor("xT", [C, T], F32, kind="ExternalInput").ap(),
        "wq": nc.dram_tensor("wq", [C, HPC * 128], F32, kind="ExternalInput").ap(),
        "wk": nc.dram_tensor("wk", [C, KVPC * 128], F32, kind="ExternalInput").ap(),
        "wv": nc.dram_tensor("wv", [C, KVPC * 128], F32, kind="ExternalInput").ap(),
        "wc": nc.dram_tensor("wc", [HPC * 128, C], F32, kind="ExternalInput").ap(),
        "masks": nc.dram_tensor("masks", [4, 128, TQ], BF16, kind="ExternalInput").ap(),
        "ones": nc.dram_tensor("ones", [128, 1], F32, kind="ExternalInput").ap(),
        "ident": nc.dram_tensor("ident", [128, 128], F32, kind="ExternalInput").ap(),
        "out": nc.dram_tensor("out", [T, C], F32, kind="ExternalOutput").ap(),
    }
    with tile.TileContext(nc) as tc:
        _emit(tc, io)
    nc.compile()
    return nc


def _get_nc():
    global _NC
    if _NC is None:
        _NC = _build_nc()
    return _NC


def make_in_maps(x, Wq, Wkv, Wc):
    x = np.asarray(x, dtype=np.float32)
    Wq = np.asarray(Wq, dtype=np.float32)
    Wkv = np.asarray(Wkv, dtype=np.float32)
    Wc = np.asarray(Wc, dtype=np.float32)
    masks = _make_masks()
    ones = np.ones((128, 1), dtype=np.float32)
    in_maps = []
    for core in range(NCORES):
        b, s = core // 2, core % 2
        in_maps.append({
            "xT": _round_f32r(x[b].T),
            "wq": _round_f32r(Wq[:, s * 1024:(s + 1) * 1024]),
            "wk": _round_f32r(Wkv[:, s * 256:(s + 1) * 256]),
            "wv": _round_f32r(Wkv[:, 512 + s * 256:512 + (s + 1) * 256]),
            "wc": _round_f32r(Wc[s * 1024:(s + 1) * 1024, :]),
            "masks": masks,
            "ones": ones,
            "ident": np.eye(128, dtype=np.float32),
        })
    return in_maps


def combine_outputs(results, bc):
    bc = np.asarray(bc, dtype=np.float32)
    out = np.empty((B, T, C), dtype=np.float32)
    for b in range(B):
        out[b] = results[2 * b]["out"] + results[2 * b + 1]["out"]
    out += bc[None, None, :]
    return out


def kernel(x, Wq, Wkv, Wc, bc):
    from concourse.bass_utils import run_bass_kernel_spmd

    nc = _get_nc()
    in_maps = make_in_maps(x, Wq, Wkv, Wc)
    res = run_bass_kernel_spmd(nc, in_maps, list(range(NCORES)))
    return combine_outputs(res.results, bc)
